# revision 1
# baseline (speedup 1.0000x reference)
"""BitNet FFN Trainium2 kernel (8-core SPMD, data-parallel over tokens).

Math (forward values of the STE reference):
  wq(w)  = clip(round(w/s), -1, 1) * s,  s = mean(|w|) + EPS        (ternary)
  xq(x)  = round(x/sx) * sx,  sx = max(absmax_row(x), EPS)/127      (int8 range)
  gate = sigmoid(xq @ wq_g.T); up = xq @ wq_u.T; h = gate*up
  out  = hq(h) @ wq_d.T

Strategy: every matmul runs in bf16 with fp32 PSUM accumulation on exact
integers (|int| <= 127 activations, ternary weights, partial sums < 2^24),
so the integer matmuls are exact; all scales are folded in fp32 outside the
matmuls. Tokens are sharded 8 ways (1024/core); each core streams the full
weights once. The only collective is a 16-byte AllReduce for the three
global weight-scale sums.
"""

import sys

sys.path.insert(0, "/opt/trn_rl_repo")

import numpy as np

import concourse.tile as tile
from concourse import bacc, mybir

F32 = mybir.dt.float32
BF16 = mybir.dt.bfloat16
ADD = mybir.AluOpType.add
SUB = mybir.AluOpType.subtract
MULT = mybir.AluOpType.mult
MAX = mybir.AluOpType.max
AXX = mybir.AxisListType.X
AFT = mybir.ActivationFunctionType

EPS = 1e-5
CR = 12582912.0  # 1.5*2^23: fp32 RNE round-to-integer magic constant
ALPHA = 1.0986122886681098  # atanh(0.5)/0.5 : tanh(ALPHA*0.5) == 0.5
P = 128


def build_program(T, DM, FF, ncores, ff_sh, dm_sh):
    """Build the per-core SPMD program.

    T: tokens per core; DM: d_model; FF: d_ff; ff_sh/dm_sh: rows of the
    per-core weight-scale shards (w_gate/w_up shard rows, w_down shard rows).
    """
    assert T % P == 0 and DM % P == 0 and FF % 1024 == 0
    MT = T // P              # token tiles
    KD = DM // P             # d_model k-blocks
    NG = FF // 1024          # phase-1 ff groups (8 strips each)
    K3 = FF // P             # phase-3 ff k-blocks
    MD = DM // P             # output dm blocks
    TN = min(512, T)         # moving free dim (tokens) per matmul
    NT3 = T // TN            # phase-3 token chunks
    WPC = min(2048, DM)      # scale-pass piece width for g/u
    WPC3 = min(2048, FF)     # scale-pass piece width for wd

    nc = bacc.Bacc(
        "TRN2",
        target_bir_lowering=False,
        debug=False,
        enable_asserts=False,
        num_devices=ncores,
    )

    x_d = nc.dram_tensor("x", [T, DM], F32, kind="ExternalInput")
    wg_d = nc.dram_tensor("wg", [FF, DM], F32, kind="ExternalInput")
    wu_d = nc.dram_tensor("wu", [FF, DM], F32, kind="ExternalInput")
    wd_d = nc.dram_tensor("wd", [DM, FF], F32, kind="ExternalInput")
    wgs_d = nc.dram_tensor("wg_sh", [ff_sh, DM], F32, kind="ExternalInput")
    wus_d = nc.dram_tensor("wu_sh", [ff_sh, DM], F32, kind="ExternalInput")
    wds_d = nc.dram_tensor("wd_sh", [dm_sh, FF], F32, kind="ExternalInput")
    out_d = nc.dram_tensor("out_t", [DM, T], F32, kind="ExternalOutput")

    NW = float(FF * DM)  # elements per weight matrix (all three equal)

    with tile.TileContext(nc, num_cores=ncores) as tc:
        import contextlib

        with contextlib.ExitStack() as outer:
            dram = outer.enter_context(tc.tile_pool(name="dram", bufs=1, space="DRAM"))
            psum = outer.enter_context(tc.tile_pool(name="psum", bufs=8, space="PSUM"))
            tiny = outer.enter_context(tc.tile_pool(name="tiny", bufs=1))

            hp_d = dram.tile([T, FF], F32)       # h' = sigmoid(G)*U_int
            shs_d = dram.tile([1, T], F32)       # per-token output scale row
            cc_in = dram.tile([1, 4], F32)
            cc_out = dram.tile([1, 4], F32)

            # persistent small tiles
            ones_col = tiny.tile([P, 1], F32)
            nc.vector.memset(ones_col, 1.0)
            ones_row = tiny.tile([1, P], F32)
            nc.vector.memset(ones_row, 1.0)
            sb_scales = tiny.tile([P, 8], F32)   # bcast: bg,bu,bd,swg,swu,swd
            sx_all = tiny.tile([P, MT], F32)     # per-token x scale (col=token tile)
            rx_all = tiny.tile([P, MT], F32)
            sxg_all = tiny.tile([P, MT], F32)    # sx*swg (sigmoid input scale)
            sxu_all = tiny.tile([P, MT], F32)    # sx*swu
            rph_all = tiny.tile([P, MT], F32)    # s_xu/s_h (h' quant scale)
            shd_all = tiny.tile([P, MT], F32)    # s_h*s_wd (output scale)
            accs = tiny.tile([P, MT, 2 * NG], F32)  # h' absmax partials

            # ---------------- S0: global weight scales ----------------
            with tc.tile_pool(name="s0", bufs=3) as s0p, tc.tile_pool(
                name="s0t", bufs=4
            ) as s0t:
                acc3 = tiny.tile([P, 4], F32)
                nc.vector.memset(acc3, 0.0)
                shard_specs = [
                    (wgs_d, 0, ff_sh, DM, WPC),
                    (wus_d, 1, ff_sh, DM, WPC),
                    (wds_d, 2, dm_sh, FF, WPC3),
                ]
                for src, col, rows, cols, pw in shard_specs:
                    for r0 in range(0, rows, P):
                        pr = min(P, rows - r0)
                        for c0 in range(0, cols, pw):
                            t_in = s0p.tile([P, pw], F32, name="s0raw")
                            nc.sync.dma_start(
                                t_in[:pr], src[r0 : r0 + pr, c0 : c0 + pw]
                            )
                            t_abs = s0p.tile([P, pw], F32, name="s0abs")
                            t_sum = s0t.tile([P, 1], F32, name="s0sum")
                            nc.scalar.activation(
                                out=t_abs[:pr],
                                in_=t_in[:pr],
                                func=AFT.Abs,
                                accum_out=t_sum[:pr],
                            )
                            nc.vector.tensor_tensor(
                                out=acc3[:pr, col : col + 1],
                                in0=acc3[:pr, col : col + 1],
                                in1=t_sum[:pr],
                                op=ADD,
                            )
                ps_s = psum.tile([P, 512], F32, name="ps_main")
                nc.tensor.matmul(
                    ps_s[:4, :1], acc3[:, :4], ones_col, start=True, stop=True
                )
                sb_s = s0t.tile([4, 1], F32, name="sb_s")
                nc.vector.tensor_copy(sb_s, ps_s[:4, :1])
                nc.sync.dma_start(cc_in[0, :4], sb_s[:, 0])
                nc.gpsimd.collective_compute(
                    "AllReduce",
                    ADD,
                    replica_groups=[list(range(ncores))],
                    ins=[cc_in[:].opt()],
                    outs=[cc_out[:].opt()],
                )
                sums_row = s0t.tile([1, 4], F32, name="sums_row")
                nc.sync.dma_start(sums_row, cc_out[:])
                sw_row = s0t.tile([1, 4], F32, name="sw_row")
                nc.vector.tensor_scalar(
                    out=sw_row, in0=sums_row, scalar1=1.0 / NW, scalar2=EPS,
                    op0=MULT, op1=ADD,
                )
                beta_row = s0t.tile([1, 4], F32, name="beta_row")
                nc.vector.reciprocal(beta_row, sw_row)
                row8 = s0t.tile([1, 8], F32, name="row8")
                nc.vector.tensor_scalar(
                    out=row8[:, 0:4], in0=beta_row, scalar1=ALPHA, scalar2=None,
                    op0=MULT, op1=mybir.AluOpType.bypass,
                )
                nc.vector.tensor_copy(row8[:, 4:8], sw_row)
                ps_b = psum.tile([P, 512], F32, name="ps_main")
                nc.tensor.matmul(
                    ps_b[:, :8], ones_row, row8, start=True, stop=True
                )
                nc.vector.tensor_copy(sb_scales, ps_b[:, :8])

            # ---------------- phase 0/1: x-quant + gate/up + h' ----------------
            with contextlib.ExitStack() as ph1:
                xqt_p = ph1.enter_context(tc.tile_pool(name="xqt", bufs=1))

                xqt = xqt_p.tile([P, KD, T], BF16)  # XqT: [dm-part, k, token]

                # x quantization (per token-tile) in its own pool scope
                with tc.tile_pool(name="xw", bufs=3) as xw_p:
                    for m in range(MT):
                        xt = xw_p.tile([P, DM], F32, name="xt")
                        nc.gpsimd.dma_start(xt, x_d[m * P : (m + 1) * P, :])
                        amax = xw_p.tile([P, 1], F32, name="amax")
                        nc.vector.tensor_reduce(
                            amax, xt, axis=AXX, op=MAX, apply_absolute_value=True
                        )
                        nc.vector.tensor_scalar(
                            out=sx_all[:, m : m + 1], in0=amax, scalar1=EPS,
                            scalar2=1.0 / 127.0, op0=MAX, op1=MULT,
                        )
                        nc.vector.reciprocal(
                            rx_all[:, m : m + 1], sx_all[:, m : m + 1]
                        )
                        nc.vector.tensor_tensor(
                            out=sxg_all[:, m : m + 1], in0=sx_all[:, m : m + 1],
                            in1=sb_scales[:, 4:5], op=MULT,
                        )
                        nc.vector.tensor_tensor(
                            out=sxu_all[:, m : m + 1], in0=sx_all[:, m : m + 1],
                            in1=sb_scales[:, 5:6], op=MULT,
                        )
                        xr = xw_p.tile([P, DM], F32, name="xr")
                        nc.vector.tensor_scalar(
                            out=xr, in0=xt, scalar1=rx_all[:, m : m + 1], scalar2=CR,
                            op0=MULT, op1=ADD,
                        )
                        xq = xw_p.tile([P, DM], BF16, name="xq")
                        nc.vector.tensor_scalar(
                            out=xq, in0=xr, scalar1=CR, scalar2=None,
                            op0=SUB, op1=mybir.AluOpType.bypass,
                        )
                        nc.sync.dma_start_transpose(
                            xqt[:, :, m * P : (m + 1) * P], xq
                        )

                wraw_p = ph1.enter_context(tc.tile_pool(name="wraw", bufs=3))
                wtern_p = ph1.enter_context(tc.tile_pool(name="wtern", bufs=3))
                wchunk_p = ph1.enter_context(tc.tile_pool(name="wchunk", bufs=6))
                gate_p = ph1.enter_context(
                    tc.tile_pool(name="gate", bufs=4)
                )
                hpr_p = ph1.enter_context(tc.tile_pool(name="hpr", bufs=2))
                sc_p = ph1.enter_context(tc.tile_pool(name="scp", bufs=2))

                # Merged gate+up pass per 512-ff group (4 strips each).
                # Ternary chunks are strip-major [P, strip(4), k(KD), 128] so
                # each strip transpose lands contiguous. One LDWEIGHTS (xqT
                # tile) feeds the G and U matmuls; 2 PSUM banks per token
                # tile so four token tiles pipeline.
                def produce_chunk(eng, wsrc, beta_col, ng):
                    chunk = wchunk_p.tile([P, 4, KD, P], BF16, name="wchunk")
                    for s4 in range(4):
                        r0 = (ng * 4 + s4) * P
                        raw = wraw_p.tile([P, DM], F32, name="wraw")
                        nc.gpsimd.dma_start(raw, wsrc[r0 : r0 + P, :])
                        nc.scalar.activation(
                            out=raw, in_=raw, func=AFT.Tanh,
                            scale=sb_scales[:, beta_col : beta_col + 1],
                        )
                        tern = wtern_p.tile([P, DM], BF16, name="wtern")
                        nc.vector.tensor_scalar(
                            out=tern, in0=raw, scalar1=CR, scalar2=CR,
                            op0=ADD, op1=SUB,
                        )
                        eng.dma_start_transpose(
                            chunk[:, s4 : s4 + 1, :, :], tern
                        )
                    return chunk

                NG5 = FF // 512
                for ng in range(NG5):
                    chunk_g = produce_chunk(nc.sync, wg_d, 0, ng)
                    chunk_u = produce_chunk(nc.sync, wu_d, 1, ng)
                    for m in range(MT):
                        psg = psum.tile([P, 512], F32, name="ps_main")
                        psu = psum.tile([P, 512], F32, name="ps_main")
                        for k in range(KD):
                            lhsT = xqt[:, k, m * P : (m + 1) * P]
                            st, sp = (k == 0), (k == KD - 1)
                            nc.tensor.matmul(
                                psg, lhsT, chunk_g[:, :, k, :], start=st, stop=sp
                            )
                            nc.tensor.matmul(
                                psu, lhsT, chunk_u[:, :, k, :], start=st, stop=sp
                            )
                        gt = gate_p.tile([P, 512], F32, name="gate_t")
                        nc.scalar.activation(
                            out=gt, in_=psg, func=AFT.Sigmoid,
                            scale=sxg_all[:, m : m + 1],
                        )
                        hp = hpr_p.tile([P, 512], F32, name="hp")
                        nc.vector.tensor_tensor(out=hp, in0=gt, in1=psu, op=MULT)
                        nc.vector.tensor_reduce(
                            accs[:, m, ng : ng + 1], hp, axis=AXX,
                            op=MAX, apply_absolute_value=True,
                        )
                        nc.scalar.dma_start(
                            hp_d[m * P : (m + 1) * P, ng * 512 : (ng + 1) * 512],
                            hp,
                        )

                # h scales per token tile
                for m in range(MT):
                    am = sc_p.tile([P, 1], F32, name="am")
                    nc.vector.tensor_reduce(
                        am, accs[:, m, :], axis=AXX, op=MAX
                    )
                    nc.vector.tensor_tensor(
                        out=am, in0=am, in1=sxu_all[:, m : m + 1], op=MULT
                    )
                    sh = sc_p.tile([P, 1], F32, name="sh")
                    nc.vector.tensor_scalar(
                        out=sh, in0=am, scalar1=EPS, scalar2=1.0 / 127.0,
                        op0=MAX, op1=MULT,
                    )
                    rs = sc_p.tile([P, 1], F32, name="rs")
                    nc.vector.reciprocal(rs, sh)
                    nc.vector.tensor_tensor(
                        out=rph_all[:, m : m + 1], in0=rs,
                        in1=sxu_all[:, m : m + 1], op=MULT,
                    )
                    nc.vector.tensor_tensor(
                        out=shd_all[:, m : m + 1], in0=sh,
                        in1=sb_scales[:, 6:7], op=MULT,
                    )
                    nc.sync.dma_start(
                        shs_d[0, m * P : (m + 1) * P], shd_all[:, m : m + 1]
                    )

            # ---------------- phase 2/3: quantize h' + down projection ----------------
            with contextlib.ExitStack() as ph23:
                hqtb_p = ph23.enter_context(tc.tile_pool(name="hqtb", bufs=1))
                # hqt: [ff-in-block, ff-block k, token] — transposed quantized h
                hqt = hqtb_p.tile([P, K3, T], BF16)

                # S5: quantize h' into hqt, ff-column-major so phase-3 matmuls
                # can consume early k columns while later ones still quantize
                with tc.tile_pool(name="s5", bufs=6) as s5p:
                    PW5 = min(2048, FF)
                    for c0 in range(0, FF, PW5):
                        for m in range(MT):
                            hpt = s5p.tile([P, PW5], F32, name="hpt")
                            nc.gpsimd.dma_start(
                                hpt, hp_d[m * P : (m + 1) * P, c0 : c0 + PW5]
                            )
                            nc.vector.tensor_scalar(
                                out=hpt, in0=hpt, scalar1=rph_all[:, m : m + 1],
                                scalar2=CR, op0=MULT, op1=ADD,
                            )
                            hqq = s5p.tile([P, PW5], BF16, name="hqq")
                            nc.vector.tensor_scalar(
                                out=hqq, in0=hpt, scalar1=CR, scalar2=None,
                                op0=SUB, op1=mybir.AluOpType.bypass,
                            )
                            nc.sync.dma_start_transpose(
                                hqt[
                                    :,
                                    c0 // P : (c0 + PW5) // P,
                                    m * P : (m + 1) * P,
                                ],
                                hqq,
                            )

                shs_p = ph23.enter_context(tc.tile_pool(name="shsp", bufs=1))
                wdr_p = ph23.enter_context(tc.tile_pool(name="wdr", bufs=2))
                wdtern_p = ph23.enter_context(tc.tile_pool(name="wdtn", bufs=1))
                wdt_p = ph23.enter_context(tc.tile_pool(name="wdtg", bufs=3))
                fin_p = ph23.enter_context(tc.tile_pool(name="finp", bufs=2))

                shs_row = shs_p.tile([1, T], F32, name="shs_row")
                nc.sync.dma_start(shs_row, shs_d[:])
                shs_bc = shs_p.tile([P, T], F32, name="shs_bc")
                for t in range(NT3):
                    ps_bc = psum.tile([P, 512], F32, name="ps_main")
                    nc.tensor.matmul(
                        ps_bc[:, :TN], ones_row,
                        shs_row[:, t * TN : (t + 1) * TN], start=True, stop=True,
                    )
                    nc.vector.tensor_copy(
                        shs_bc[:, t * TN : (t + 1) * TN], ps_bc[:, :TN]
                    )

                # fused: ternarize+transpose w_down per output dm-block,
                # full-k PSUM accumulation; emitted inside the s5 scope so
                # wd production and early matmuls overlap quantization
                KH = K3 // 2  # k-blocks per wdtg half-tile
                for md in range(MD):
                    halves = []
                    for h in range(2):
                        wdtg = wdt_p.tile([P, KH, P], BF16, name="wdtg")
                        halves.append(wdtg)
                        base = h * (FF // 2)
                        PW3 = min(2048, FF // 2)
                        for c0 in range(0, FF // 2, PW3):
                            raw = wdr_p.tile([P, PW3], F32, name="wdraw")
                            nc.gpsimd.dma_start(
                                raw,
                                wd_d[
                                    md * P : (md + 1) * P,
                                    base + c0 : base + c0 + PW3,
                                ],
                            )
                            nc.scalar.activation(
                                out=raw, in_=raw, func=AFT.Tanh,
                                scale=sb_scales[:, 2:3],
                            )
                            ternd = wdtern_p.tile([P, PW3], BF16, name="wdtern")
                            nc.vector.tensor_scalar(
                                out=ternd, in0=raw, scalar1=CR, scalar2=CR,
                                op0=ADD, op1=SUB,
                            )
                            nc.sync.dma_start_transpose(
                                wdtg[:, c0 // P : (c0 + PW3) // P, :], ternd
                            )
                    pss = [
                        psum.tile([P, 512], F32, name="ps_main")
                        for _ in range(NT3)
                    ]
                    for k in range(K3):
                        lhsT = halves[k // KH][:, k % KH, :]
                        for t in range(NT3):
                            nc.tensor.matmul(
                                pss[t][:, :TN],
                                lhsT,
                                hqt[:, k, t * TN : (t + 1) * TN],
                                start=(k == 0),
                                stop=(k == K3 - 1),
                            )
                    for t in range(NT3):
                        ot = fin_p.tile([P, TN], F32, name="ot")
                        nc.vector.tensor_tensor(
                            out=ot, in0=pss[t][:, :TN],
                            in1=shs_bc[:, t * TN : (t + 1) * TN], op=MULT,
                        )
                        nc.scalar.dma_start(
                            out_d[md * P : (md + 1) * P, t * TN : (t + 1) * TN],
                            ot,
                        )

    nc.compile()
    return nc


_CACHE = {}
TRACE = False  # set True (e.g. from test.py) to capture an NTFF profile
LAST_RESULTS = None


def _get_program(T, DM, FF, ncores, ff_sh, dm_sh):
    key = (T, DM, FF, ncores, ff_sh, dm_sh)
    if key not in _CACHE:
        _CACHE[key] = build_program(T, DM, FF, ncores, ff_sh, dm_sh)
    return _CACHE[key]


def kernel(x, w_gate, w_up, w_down):
    from concourse.bass_utils import run_bass_kernel_spmd

    x = np.asarray(x, dtype=np.float32)
    w_gate = np.ascontiguousarray(np.asarray(w_gate, dtype=np.float32))
    w_up = np.ascontiguousarray(np.asarray(w_up, dtype=np.float32))
    w_down = np.ascontiguousarray(np.asarray(w_down, dtype=np.float32))

    B, S, DM = x.shape
    FF = w_gate.shape[0]
    NCORES = 8
    NTOK = B * S
    T = NTOK // NCORES
    ff_sh = FF // NCORES
    dm_sh = DM // NCORES

    xf = np.ascontiguousarray(x.reshape(NTOK, DM))
    nc = _get_program(T, DM, FF, NCORES, ff_sh, dm_sh)

    in_maps = []
    for c in range(NCORES):
        in_maps.append(
            {
                "x": np.ascontiguousarray(xf[c * T : (c + 1) * T]),
                "wg": w_gate,
                "wu": w_up,
                "wd": w_down,
                "wg_sh": np.ascontiguousarray(w_gate[c * ff_sh : (c + 1) * ff_sh]),
                "wu_sh": np.ascontiguousarray(w_up[c * ff_sh : (c + 1) * ff_sh]),
                "wd_sh": np.ascontiguousarray(w_down[c * dm_sh : (c + 1) * dm_sh]),
            }
        )

    res = run_bass_kernel_spmd(
        nc, in_maps, core_ids=list(range(NCORES)), trace=TRACE
    )
    global LAST_RESULTS
    LAST_RESULTS = res
    out = np.empty((NTOK, DM), dtype=np.float32)
    for c in range(NCORES):
        out[c * T : (c + 1) * T] = res.results[c]["out_t"].T
    return out.reshape(B, S, DM)



# revision 30
# speedup vs baseline: 1.0077x; 1.0077x over previous
"""BitNet FFN Trainium2 kernel (8-core SPMD, data-parallel over tokens).

Math (forward values of the STE reference):
  wq(w)  = clip(round(w/s), -1, 1) * s,  s = mean(|w|) + EPS        (ternary)
  xq(x)  = round(x/sx) * sx,  sx = max(absmax_row(x), EPS)/127      (int8 range)
  gate = sigmoid(xq @ wq_g.T); up = xq @ wq_u.T; h = gate*up
  out  = hq(h) @ wq_d.T

Every matmul runs in bf16 with fp32 PSUM accumulation on exact integers
(|int| <= 127 activations, ternary weights, partial sums < 2^24), so the
integer matmuls are exact; all scales are folded in fp32 outside the
matmuls.  Tokens are sharded 8 ways (1024/core); each core streams the
full weights once.  The only collective is a 16-byte AllReduce for the
three global weight-scale sums.

Phase-1 produces h' directly in [ff, tok] layout (stationary operand =
transposed ternary weight block, moving operand = transposed quantized
activations), so the phase-3 contraction input hqt needs no transpose:
h' is spilled to DRAM fp32 and read back contiguously.  Per-token scales
live as broadcast rows [P, T].
"""

import sys

sys.path.insert(0, "/opt/trn_rl_repo")

import numpy as np

import concourse.tile as tile
from concourse import bacc, mybir, bass_isa

F32 = mybir.dt.float32
BF16 = mybir.dt.bfloat16
ADD = mybir.AluOpType.add
SUB = mybir.AluOpType.subtract
MULT = mybir.AluOpType.mult
MAX = mybir.AluOpType.max
ABSMAX = mybir.AluOpType.abs_max
BYPASS = mybir.AluOpType.bypass
AXX = mybir.AxisListType.X
AFT = mybir.ActivationFunctionType
RED = bass_isa.ReduceOp

EPS = 1e-5
CR = 12582912.0  # 1.5*2^23: fp32 RNE round-to-integer magic constant
ALPHA = 1.0986122886681098  # atanh(0.5)/0.5 : tanh(ALPHA*0.5) == 0.5
P = 128


def build_program(T, DM, FF, ncores, ff_sh, dm_sh):
    """Build the per-core SPMD program.

    T: tokens per core; DM: d_model; FF: d_ff; ff_sh/dm_sh: rows of the
    per-core weight-scale shards (w_gate/w_up shard rows, w_down shard rows).
    """
    assert T % P == 0 and DM % P == 0 and FF % P == 0
    MT = T // P              # token tiles
    KD = DM // P             # d_model k-blocks
    NFB = FF // P            # ff blocks (phase-1 output blocks / phase-3 k)
    MD = DM // P             # output dm blocks
    TCH = min(512, T)        # token chunk (psum free dim)
    NTC = T // TCH           # token chunks
    KW3 = min(1024, FF)      # phase-3 wd piece width
    NW3 = FF // KW3          # wd pieces per md
    KB3 = KW3 // P           # k-blocks per wd piece

    nc = bacc.Bacc(
        "TRN2",
        target_bir_lowering=False,
        debug=False,
        enable_asserts=False,
        num_devices=ncores,
    )

    x_d = nc.dram_tensor("x", [T, DM], F32, kind="ExternalInput")
    wg_d = nc.dram_tensor("wg", [FF, DM], F32, kind="ExternalInput")
    wu_d = nc.dram_tensor("wu", [FF, DM], F32, kind="ExternalInput")
    wd_d = nc.dram_tensor("wd", [DM, FF], F32, kind="ExternalInput")
    wgs_d = nc.dram_tensor("wg_sh", [ff_sh, DM], F32, kind="ExternalInput")
    wus_d = nc.dram_tensor("wu_sh", [ff_sh, DM], F32, kind="ExternalInput")
    wds_d = nc.dram_tensor("wd_sh", [dm_sh, FF], F32, kind="ExternalInput")
    out_d = nc.dram_tensor("out_t", [DM, T], F32, kind="ExternalOutput")

    NW = float(FF * DM)  # elements per weight matrix (all three equal)

    with tile.TileContext(nc, num_cores=ncores) as tc:
        import contextlib

        with contextlib.ExitStack() as outer:
            dram = outer.enter_context(tc.tile_pool(name="dram", bufs=1, space="DRAM"))
            psum = outer.enter_context(tc.tile_pool(name="psum", bufs=8, space="PSUM"))
            tiny = outer.enter_context(tc.tile_pool(name="tiny", bufs=1))
            # rph/shd broadcast rows survive into phase 3
            bc2_p = outer.enter_context(tc.tile_pool(name="bc2", bufs=2))

            hp_d = dram.tile([NFB, NTC, P, TCH], F32)  # h' blocked fp32
            sx_d = dram.tile([1, T], F32)              # per-token x scale row
            rph_d = dram.tile([1, T], F32)             # per-token h quant scale
            shd_d = dram.tile([1, T], F32)             # per-token out scale
            cc_in = dram.tile([1, 4], F32)
            cc_out = dram.tile([1, 4], F32)

            sb_scales = tiny.tile([P, 8], F32)   # bcast: bg,bu,bd,-,swg,swu,swd,-
            sx_all = tiny.tile([P, MT], F32)     # per-token x scale (col=tile)
            rx_all = tiny.tile([P, MT], F32)
            sxu_all = tiny.tile([P, MT], F32)    # sx*swu columns
            ones_col = tiny.tile([P, 1], F32)
            nc.vector.memset(ones_col, 1.0)
            ones_row = tiny.tile([1, P], F32)
            nc.vector.memset(ones_row, 1.0)
            ident = tiny.tile([P, P], F32)       # for PE transpose
            nc.vector.memset(ident, 1.0)
            nc.gpsimd.affine_select(
                out=ident, in_=ident, pattern=[[1, P]],
                compare_op=mybir.AluOpType.is_equal, fill=0.0,
                base=0, channel_multiplier=-1,
            )

            def pe_broadcast_row(row, dst):
                """dst[P, T] = broadcast of row [1, T] to all partitions."""
                for c0 in range(0, T, TCH):
                    psb = psum.tile([P, TCH], F32, name="ps_main")
                    nc.tensor.matmul(
                        psb, ones_row, row[:, c0 : c0 + TCH],
                        start=True, stop=True,
                    )
                    nc.vector.tensor_copy(dst[:, c0 : c0 + TCH], psb)

            # ---------------- S0: global weight scales ----------------
            with tc.tile_pool(name="s0", bufs=6) as s0p, tc.tile_pool(
                name="s0t", bufs=8
            ) as s0t:
                acc3 = tiny.tile([P, 4], F32)
                nc.vector.memset(acc3, 0.0)
                # (src, acc col, rows, cols, piece width, load engine, abs engine)
                shard_specs = [
                    (wgs_d, 0, ff_sh, DM, min(2048, DM), nc.sync),
                    (wus_d, 1, ff_sh, DM, min(2048, DM), nc.scalar),
                    (wds_d, 2, dm_sh, FF, min(2048, FF), nc.gpsimd),
                ]
                for src, col, rows, cols, pw, ldeng in shard_specs:
                    for r0 in range(0, rows, P):
                        pr = min(P, rows - r0)
                        for c0 in range(0, cols, pw):
                            t_in = s0p.tile([P, pw], F32, name="s0raw")
                            ldeng.dma_start(
                                t_in[:pr], src[r0 : r0 + pr, c0 : c0 + pw]
                            )
                            t_sum = s0t.tile([P, 1], F32, name="s0sum")
                            t_abs = s0p.tile([P, pw], F32, name="s0abs")
                            nc.scalar.activation(
                                out=t_abs[:pr],
                                in_=t_in[:pr],
                                func=AFT.Abs,
                                accum_out=t_sum[:pr],
                            )
                            nc.vector.tensor_tensor(
                                out=acc3[:pr, col : col + 1],
                                in0=acc3[:pr, col : col + 1],
                                in1=t_sum[:pr],
                                op=ADD,
                            )
                # cross-partition sum + 16B AllReduce for global sums
                ps_s = psum.tile([P, TCH], F32, name="ps_main")
                nc.tensor.matmul(
                    ps_s[:4, :1], acc3[:, :4], ones_col, start=True, stop=True
                )
                sb_s = s0t.tile([4, 1], F32, name="sb_s")
                nc.vector.tensor_copy(sb_s, ps_s[:4, :1])
                nc.sync.dma_start(cc_in[0, :4], sb_s[:, 0])
                nc.gpsimd.collective_compute(
                    "AllReduce",
                    ADD,
                    replica_groups=[list(range(ncores))],
                    ins=[cc_in[:].opt()],
                    outs=[cc_out[:].opt()],
                )
                sums_row = s0t.tile([1, 4], F32, name="sums_row")
                nc.sync.dma_start(sums_row, cc_out[:])
                sw_row = s0t.tile([1, 4], F32, name="sw_row")
                nc.vector.tensor_scalar(
                    out=sw_row, in0=sums_row, scalar1=1.0 / NW, scalar2=EPS,
                    op0=MULT, op1=ADD,
                )
                beta_row = s0t.tile([1, 4], F32, name="beta_row")
                nc.vector.reciprocal(beta_row, sw_row)
                row8 = s0t.tile([1, 8], F32, name="row8")
                nc.vector.tensor_scalar(
                    out=row8[:, 0:4], in0=beta_row, scalar1=ALPHA, scalar2=None,
                    op0=MULT, op1=BYPASS,
                )
                nc.vector.tensor_copy(row8[:, 4:8], sw_row)
                ps_b = psum.tile([P, TCH], F32, name="ps_main")
                nc.tensor.matmul(
                    ps_b[:, :8], ones_row, row8, start=True, stop=True
                )
                nc.vector.tensor_copy(sb_scales, ps_b[:, :8])

            # ---------------- phase 1: x-quant + gate/up -> h' [ff,tok] ----
            with contextlib.ExitStack() as ph1:
                xqt_p = ph1.enter_context(tc.tile_pool(name="xqt", bufs=1))
                xqt = xqt_p.tile([P, KD, T], BF16)  # XqT: [dm-part, k, token]
                acc_p = ph1.enter_context(tc.tile_pool(name="accp", bufs=1))
                acc_hi = acc_p.tile([P, T], F32)    # h' max partials (per tok)
                acc_lo = acc_p.tile([P, T], F32)    # h' min partials (per tok)
                nc.vector.memset(acc_hi, 0.0)
                nc.vector.memset(acc_lo, 0.0)
                bc_p = ph1.enter_context(tc.tile_pool(name="bc", bufs=4))

                # x quantization (per token-tile)
                with tc.tile_pool(name="xw", bufs=4) as xw_p:
                    for m in range(MT):
                        xt = xw_p.tile([P, DM], F32, name="xt")
                        nc.gpsimd.dma_start(xt, x_d[m * P : (m + 1) * P, :])
                        amax = xw_p.tile([P, 1], F32, name="amax")
                        nc.vector.tensor_reduce(
                            amax, xt, axis=AXX, op=MAX, apply_absolute_value=True
                        )
                        nc.vector.tensor_scalar(
                            out=sx_all[:, m : m + 1], in0=amax, scalar1=EPS,
                            scalar2=1.0 / 127.0, op0=MAX, op1=MULT,
                        )
                        nc.vector.reciprocal(
                            rx_all[:, m : m + 1], sx_all[:, m : m + 1]
                        )
                        xr = xw_p.tile([P, DM], F32, name="xr")
                        nc.vector.tensor_scalar(
                            out=xr, in0=xt, scalar1=rx_all[:, m : m + 1], scalar2=CR,
                            op0=MULT, op1=ADD,
                        )
                        xq = xw_p.tile([P, DM], BF16, name="xq")
                        nc.vector.tensor_scalar(
                            out=xq, in0=xr, scalar1=CR, scalar2=None,
                            op0=SUB, op1=BYPASS,
                        )
                        nc.sync.dma_start_transpose(
                            xqt[:, :, m * P : (m + 1) * P], xq
                        )
                        # stash sx column into the [1,T] row for broadcasting
                        nc.sync.dma_start(
                            sx_d[0, m * P : (m + 1) * P], sx_all[:, m : m + 1]
                        )

                # per-token scale broadcast rows [P, T]
                sx_row = bc_p.tile([1, T], F32, name="bc_row")
                nc.sync.dma_start(sx_row, sx_d[:])
                sx_bc = bc_p.tile([P, T], F32, name="bc")
                pe_broadcast_row(sx_row, sx_bc)
                sxg_bc = bc_p.tile([P, T], F32, name="bc")
                nc.vector.tensor_scalar(
                    out=sxg_bc, in0=sx_bc, scalar1=sb_scales[:, 4:5],
                    scalar2=None, op0=MULT, op1=BYPASS,
                )
                sxu_bc = bc_p.tile([P, T], F32, name="bc")
                nc.vector.tensor_scalar(
                    out=sxu_bc, in0=sx_bc, scalar1=sb_scales[:, 5:6],
                    scalar2=None, op0=MULT, op1=BYPASS,
                )
                nc.vector.tensor_scalar(
                    out=sxu_all, in0=sx_all, scalar1=sb_scales[:, 5:6],
                    scalar2=None, op0=MULT, op1=BYPASS,
                )

                wraw_p = ph1.enter_context(tc.tile_pool(name="wraw", bufs=4))
                wtern_p = ph1.enter_context(tc.tile_pool(name="wtern", bufs=4))
                wchunk_p = ph1.enter_context(tc.tile_pool(name="wchunk", bufs=15))
                gt_p = ph1.enter_context(tc.tile_pool(name="gtp", bufs=4))
                hpr_p = ph1.enter_context(tc.tile_pool(name="hpr", bufs=4))

                def produce_chunk(wsrc, beta_col, b, teng):
                    raw = wraw_p.tile([P, DM], F32, name="wraw")
                    nc.gpsimd.dma_start(raw, wsrc[b * P : (b + 1) * P, :])
                    nc.scalar.activation(
                        out=raw, in_=raw, func=AFT.Tanh,
                        scale=sb_scales[:, beta_col : beta_col + 1],
                    )
                    tern = wtern_p.tile([P, DM], BF16, name="wtern")
                    nc.vector.tensor_scalar(
                        out=tern, in0=raw, scalar1=CR, scalar2=CR,
                        op0=ADD, op1=SUB,
                    )
                    ch = wchunk_p.tile([P, KD, P], BF16, name="wchunk")
                    teng.dma_start_transpose(ch, tern)
                    return ch

                for b in range(NFB):
                    chg = produce_chunk(wg_d, 0, b, nc.sync)
                    chu = produce_chunk(wu_d, 1, b, nc.sync)
                    psg = [
                        psum.tile([P, TCH], F32, name="ps_main")
                        for _ in range(NTC)
                    ]
                    psu = [
                        psum.tile([P, TCH], F32, name="ps_main")
                        for _ in range(NTC)
                    ]
                    for k in range(KD):
                        st, sp = (k == 0), (k == KD - 1)
                        for t in range(NTC):
                            nc.tensor.matmul(
                                psg[t], chg[:, k, :],
                                xqt[:, k, t * TCH : (t + 1) * TCH],
                                start=st, stop=sp,
                            )
                        for t in range(NTC):
                            nc.tensor.matmul(
                                psu[t], chu[:, k, :],
                                xqt[:, k, t * TCH : (t + 1) * TCH],
                                start=st, stop=sp,
                            )
                    for t in range(NTC):
                        sl = slice(t * TCH, (t + 1) * TCH)
                        gt = gt_p.tile([P, TCH], F32, name="gt")
                        nc.vector.tensor_tensor(
                            out=gt, in0=psg[t], in1=sxg_bc[:, sl], op=MULT
                        )
                        nc.scalar.activation(out=gt, in_=gt, func=AFT.Sigmoid)
                        hp = hpr_p.tile([P, TCH], F32, name="hp")
                        nc.vector.tensor_tensor(
                            out=hp, in0=gt, in1=psu[t], op=MULT
                        )
                        nc.vector.tensor_tensor(
                            out=acc_hi[:, sl], in0=acc_hi[:, sl], in1=hp,
                            op=MAX,
                        )
                        nc.vector.tensor_tensor(
                            out=acc_lo[:, sl], in0=acc_lo[:, sl], in1=hp,
                            op=mybir.AluOpType.min,
                        )
                        nc.scalar.dma_start(hp_d[b, t], hp)

                # ---- h quantization scales ----
                # per-token absmax: PE-transpose the [ff-pos, tok] partials to
                # token-partition columns, reduce along free axis, then do the
                # scale math as columns [P, 1] per token tile (baseline form).
                with tc.tile_pool(name="hscl", bufs=8) as hs_p:
                    for m in range(MT):
                        msl = slice(m * P, (m + 1) * P)
                        pthi = psum.tile([P, P], F32, name="ps_main")
                        nc.tensor.transpose(pthi, acc_hi[:, msl], ident)
                        ptlo = psum.tile([P, P], F32, name="ps_main")
                        nc.tensor.transpose(ptlo, acc_lo[:, msl], ident)
                        chi = hs_p.tile([P, 1], F32, name="chi")
                        nc.vector.tensor_reduce(
                            chi, pthi, axis=AXX, op=MAX,
                            apply_absolute_value=True,
                        )
                        clo = hs_p.tile([P, 1], F32, name="clo")
                        nc.vector.tensor_reduce(
                            clo, ptlo, axis=AXX, op=MAX,
                            apply_absolute_value=True,
                        )
                        habs_c = hs_p.tile([P, 1], F32, name="habs_c")
                        nc.vector.tensor_tensor(
                            out=habs_c, in0=chi, in1=clo, op=MAX
                        )
                        sh_c = hs_p.tile([P, 1], F32, name="sh_c")
                        nc.vector.tensor_tensor(
                            out=sh_c, in0=habs_c, in1=sxu_all[:, m : m + 1],
                            op=MULT,
                        )
                        nc.vector.tensor_scalar(
                            out=sh_c, in0=sh_c, scalar1=EPS,
                            scalar2=1.0 / 127.0, op0=MAX, op1=MULT,
                        )
                        rec_c = hs_p.tile([P, 1], F32, name="rec_c")
                        nc.vector.reciprocal(rec_c, sh_c)
                        rph_c = hs_p.tile([P, 1], F32, name="rph_c")
                        nc.vector.tensor_tensor(
                            out=rph_c, in0=rec_c, in1=sxu_all[:, m : m + 1],
                            op=MULT,
                        )
                        shd_c = hs_p.tile([P, 1], F32, name="shd_c")
                        nc.vector.tensor_scalar(
                            out=shd_c, in0=sh_c, scalar1=sb_scales[:, 6:7],
                            scalar2=None, op0=MULT, op1=BYPASS,
                        )
                        nc.sync.dma_start(rph_d[0, msl], rph_c[:, 0:1])
                        nc.sync.dma_start(shd_d[0, msl], shd_c[:, 0:1])

                    rph_row = bc_p.tile([1, T], F32, name="bc_row")
                    nc.sync.dma_start(rph_row, rph_d[:])
                    rph_bc = bc2_p.tile([P, T], F32, name="bc2")
                    pe_broadcast_row(rph_row, rph_bc)
                    shd_row = bc_p.tile([1, T], F32, name="bc_row")
                    nc.sync.dma_start(shd_row, shd_d[:])
                    shd_bc = bc2_p.tile([P, T], F32, name="bc2")
                    pe_broadcast_row(shd_row, shd_bc)

            # ---------------- phase 3: quantize h' + down projection -------
            with contextlib.ExitStack() as ph3:
                hqt_p = ph3.enter_context(tc.tile_pool(name="hqt", bufs=1))
                hqt = hqt_p.tile([P, NFB, T], BF16)  # [ff-in-blk, ff-blk, tok]
                stage_p = ph3.enter_context(tc.tile_pool(name="stage", bufs=4))
                wdr_p = ph3.enter_context(tc.tile_pool(name="wdr", bufs=2))
                wdtern_p = ph3.enter_context(tc.tile_pool(name="wdtn", bufs=2))
                wdt_p = ph3.enter_context(tc.tile_pool(name="wdtg", bufs=2 * NW3))
                fin_p = ph3.enter_context(tc.tile_pool(name="finp", bufs=2))

                def quantize_block(b):
                    stage = stage_p.tile([P, T], F32, name="stage")
                    for t in range(NTC):
                        nc.sync.dma_start(
                            stage[:, t * TCH : (t + 1) * TCH], hp_d[b, t]
                        )
                    stage2 = stage_p.tile([P, T], F32, name="stage")
                    nc.vector.tensor_tensor(
                        out=stage2, in0=stage, in1=rph_bc, op=MULT
                    )
                    nc.vector.tensor_scalar(
                        out=hqt[:, b, :], in0=stage2, scalar1=CR, scalar2=CR,
                        op0=ADD, op1=SUB,
                    )

                def produce_wd(md):
                    pieces = []
                    for w in range(NW3):
                        raw = wdr_p.tile([P, KW3], F32, name="wdraw")
                        nc.gpsimd.dma_start(
                            raw,
                            wd_d[md * P : (md + 1) * P, w * KW3 : (w + 1) * KW3],
                        )
                        nc.scalar.activation(
                            out=raw, in_=raw, func=AFT.Tanh,
                            scale=sb_scales[:, 2:3],
                        )
                        ternd = wdtern_p.tile([P, KW3], BF16, name="wdtern")
                        nc.vector.tensor_scalar(
                            out=ternd, in0=raw, scalar1=CR, scalar2=CR,
                            op0=ADD, op1=SUB,
                        )
                        wdtg = wdt_p.tile([P, KB3, P], BF16, name="wdtg")
                        nc.sync.dma_start_transpose(wdtg, ternd)
                        pieces.append(wdtg)
                    return pieces

                # Emit the first mds' weight pipelines ahead of the hqt fill
                # so their vector/scalar work isn't queued behind it; all
                # hqt writers must be emitted before any consuming matmul.
                NAHEAD = min(2, MD)
                wd_pieces = {md: produce_wd(md) for md in range(NAHEAD)}
                for b in range(NFB):
                    quantize_block(b)
                for md in range(MD):
                    pieces = wd_pieces.pop(md) if md in wd_pieces else produce_wd(md)
                    pss = [
                        psum.tile([P, TCH], F32, name="ps_main")
                        for _ in range(NTC)
                    ]
                    for k in range(NFB):
                        lhsT = pieces[k // KB3][:, k % KB3, :]
                        st, sp = (k == 0), (k == NFB - 1)
                        for t in range(NTC):
                            nc.tensor.matmul(
                                pss[t], lhsT,
                                hqt[:, k, t * TCH : (t + 1) * TCH],
                                start=st, stop=sp,
                            )
                    for t in range(NTC):
                        sl = slice(t * TCH, (t + 1) * TCH)
                        ot = fin_p.tile([P, TCH], F32, name="ot")
                        nc.vector.tensor_tensor(
                            out=ot, in0=pss[t], in1=shd_bc[:, sl], op=MULT
                        )
                        nc.scalar.dma_start(
                            out_d[md * P : (md + 1) * P, sl], ot
                        )

    nc.compile()
    return nc


_CACHE = {}
TRACE = False  # set True (e.g. from test.py) to capture an NTFF profile
LAST_RESULTS = None


def _get_program(T, DM, FF, ncores, ff_sh, dm_sh):
    key = (T, DM, FF, ncores, ff_sh, dm_sh)
    if key not in _CACHE:
        _CACHE[key] = build_program(T, DM, FF, ncores, ff_sh, dm_sh)
    return _CACHE[key]


def kernel(x, w_gate, w_up, w_down):
    from concourse.bass_utils import run_bass_kernel_spmd

    x = np.asarray(x, dtype=np.float32)
    w_gate = np.ascontiguousarray(np.asarray(w_gate, dtype=np.float32))
    w_up = np.ascontiguousarray(np.asarray(w_up, dtype=np.float32))
    w_down = np.ascontiguousarray(np.asarray(w_down, dtype=np.float32))

    B, S, DM = x.shape
    FF = w_gate.shape[0]
    NCORES = 8
    NTOK = B * S
    T = NTOK // NCORES
    ff_sh = FF // NCORES
    dm_sh = DM // NCORES

    xf = np.ascontiguousarray(x.reshape(NTOK, DM))
    nc = _get_program(T, DM, FF, NCORES, ff_sh, dm_sh)

    in_maps = []
    for c in range(NCORES):
        in_maps.append(
            {
                "x": np.ascontiguousarray(xf[c * T : (c + 1) * T]),
                "wg": w_gate,
                "wu": w_up,
                "wd": w_down,
                "wg_sh": np.ascontiguousarray(w_gate[c * ff_sh : (c + 1) * ff_sh]),
                "wu_sh": np.ascontiguousarray(w_up[c * ff_sh : (c + 1) * ff_sh]),
                "wd_sh": np.ascontiguousarray(w_down[c * dm_sh : (c + 1) * dm_sh]),
            }
        )

    res = run_bass_kernel_spmd(
        nc, in_maps, core_ids=list(range(NCORES)), trace=TRACE
    )
    global LAST_RESULTS
    LAST_RESULTS = res
    out = np.empty((NTOK, DM), dtype=np.float32)
    for c in range(NCORES):
        out[c * T : (c + 1) * T] = res.results[c]["out_t"].T
    return out.reshape(B, S, DM)
